# revision 2
# baseline (speedup 1.0000x reference)
import sys

import numpy as np

sys.path.insert(0, "/opt/trn_rl_repo")

import concourse.bass as bass  # noqa: F401
import concourse.mybir as mybir
import concourse.tile as tile
from concourse import bacc
from concourse.bass_utils import run_bass_kernel_spmd

D = H = W = 128
HW = H * W
SIGMA = 3
K = 7
N_CORES = 8

_NC_CACHE = {}


def _blur_matrix(g: np.ndarray) -> np.ndarray:
    # Dense 128x128 operator for a clamped (edge-padded) 1D blur along a
    # length-128 axis: A[i, j] = sum of g[k] over taps where clamp(i+k-3)==j.
    A = np.zeros((D, D), dtype=np.float64)
    for i in range(D):
        for k in range(K):
            j = min(max(i + k - SIGMA, 0), D - 1)
            A[i, j] += float(g[k])
    return A.astype(np.float32)


def _build():
    f16 = mybir.dt.float16
    f32 = mybir.dt.float32
    nc = bacc.Bacc("TRN2", target_bir_lowering=False, debug=False)
    x = nc.dram_tensor("x", [D, HW], f16, kind="ExternalInput")
    at = nc.dram_tensor("at", [D, D], f16, kind="ExternalInput")
    out = nc.dram_tensor("out", [D, HW], f16, kind="ExternalOutput")

    # DMA-in chunk widths (cols): tapered tail so the last arrivals feed
    # small matmul/evac blocks and the R1->R2 barrier drains quickly
    CHUNKS = [2048] * 6 + [1024] * 2 + [512] * 4
    assert sum(CHUNKS) == HW
    # R1 evac blocks align with the in-chunk taper
    SIZES1 = [8] * 14 + [4, 4, 4, 4]
    # mid round: small tail blocks drain the inter-round barrier on both
    # engines in parallel
    SIZES = [8] * 15 + [4, 4]
    # last round: small head blocks too, so the first out-DMA chunk (and
    # hence the whole store stream) starts as early as possible
    SIZES3 = [4, 4] + [8] * 14 + [4, 4]
    assert sum(SIZES1) == sum(SIZES) == sum(SIZES3) == D

    with tile.TileContext(nc) as tc:
        with tc.tile_pool(name="vol", bufs=1) as vol, \
             tc.tile_pool(name="cst", bufs=1) as cst, \
             tc.tile_pool(name="ps8", bufs=4, space="PSUM") as ps8:
            att = cst.tile([D, D], f16)

            v1 = vol.tile([D, HW], f16)
            v2 = vol.tile([D, HW], f16)
            v3 = vol.tile([D, HW], f16)
            v4 = vol.tile([D, HW], f16)

            # the blur matrix rides the gpsimd SWDGE queue: it reaches the
            # DMA engines before the first x chunk without displacing it
            nc.gpsimd.dma_start(att[:], at[:])
            c0 = 0
            for cw in CHUNKS:
                nc.sync.dma_start(v1[:, c0:c0 + cw], x[:, c0:c0 + cw])
                c0 += cw

            # evac cost model (ns) for greedy engine balancing
            DVE_NS = lambda n: n * 1.0417 + 125.0
            ACT_NS = lambda n: n * 0.8333 + 185.0

            def blur_round(src, dst3, out_hbm=None, sizes=SIZES):
                # src: [p, 16384]; tile t = src[:, t*128:(t+1)*128] (stationary)
                # out[f, a'] = sum_p src[p, t, f] * AT[p, a']  -> psum [f, a']
                # dst3: [p, tile(128), a'(128)] view of the destination volume,
                # or (out_hbm set) 2D with blocks contiguous: evac + DMA out
                # per block so the store overlaps the rest of the round.
                busy = {"v": 0.0, "s": 0.0}

                def evac(dslc, src_ap, n, force=None):
                    # pick engine that would finish first
                    use_v = (busy["v"] + DVE_NS(n) <= busy["s"] + ACT_NS(n)
                             if force is None else force == "v")
                    if use_v:
                        busy["v"] += DVE_NS(n)
                        nc.vector.tensor_copy(dslc, src_ap)
                    else:
                        busy["s"] += ACT_NS(n)
                        nc.scalar.copy(dslc, src_ap)

                t0 = 0
                for i, bb in enumerate(sizes):
                    force = ("s" if i == len(sizes) - 2 else
                             "v" if i == len(sizes) - 1 else None)
                    pt = ps8.tile([D, bb * D], f32)
                    for s in range(bb):
                        t = t0 + s
                        nc.tensor.matmul(pt[:, s * D:(s + 1) * D],
                                         src[:, t * D:(t + 1) * D],
                                         att[:], start=True, stop=True)
                    if out_hbm is None:
                        evac(dst3[:, t0:t0 + bb, :],
                             pt[:].rearrange("p (t a) -> p t a", t=bb),
                             bb * D, force)
                    else:
                        lo, hi = t0 * D, (t0 + bb) * D
                        evac(dst3[:, lo:hi], pt[:], bb * D, force)
                        nc.sync.dma_start(out_hbm[:, lo:hi], dst3[:, lo:hi])
                    t0 += bb

            # R1: blur d, batch h.  v1 [d | (h w)] -> v2 [w | (dp h)]
            blur_round(v1[:], v2[:].rearrange("p (a t) -> p t a", a=D),
                       sizes=SIZES1)
            # R2: blur w, batch dp. v2 [w | (dp h)] -> v3 [h | (wp dp)]
            blur_round(v2[:], v3[:].rearrange("p (a t) -> p t a", a=D))
            # R3: blur h, batch wp. v3 [h | (wp dp)] -> v4 [dp | (wp hp)];
            # block b (wp in [8b,8b+8)) is contiguous in v4 and in out_hbm,
            # so each block streams to HBM as soon as it is evacuated.
            blur_round(v3[:], v4[:], out_hbm=out, sizes=SIZES3)
    nc.finalize()
    return nc


def kernel(x, g, sigma):
    x = np.asarray(x, dtype=np.float32)
    if "nc" not in _NC_CACHE:
        _NC_CACHE["nc"] = _build()
    nc = _NC_CACHE["nc"]
    AT = np.ascontiguousarray(_blur_matrix(np.asarray(g, np.float64)).T
                              ).astype(np.float16)
    slabs = x.reshape(N_CORES, D, HW).astype(np.float16)
    in_maps = [{"x": np.ascontiguousarray(slabs[i]), "at": AT}
               for i in range(N_CORES)]
    res = run_bass_kernel_spmd(nc, in_maps, core_ids=list(range(N_CORES)))
    outs = np.stack([res.results[i]["out"] for i in range(N_CORES)])
    # device emits [d, w, h]; swap the last two axes back to [d, h, w]
    outs = outs.reshape(N_CORES, D, W, H).astype(np.float32)
    return np.ascontiguousarray(outs.transpose(0, 1, 3, 2)
                                ).reshape(2, 4, D, H, W)


# revision 3
# speedup vs baseline: 1.0412x; 1.0412x over previous
import sys

import numpy as np

sys.path.insert(0, "/opt/trn_rl_repo")

import concourse.bass as bass  # noqa: F401
import concourse.mybir as mybir
import concourse.tile as tile
from concourse import bacc
from concourse.bass_utils import run_bass_kernel_spmd

D = H = W = 128
HW = H * W
SIGMA = 3
K = 7
N_CORES = 8

# d-split: half A covers d' in [0,64) reading x rows [0,67);
#          half B covers d' in [64,128) reading x rows [61,128)
DH = 64           # d' per half
XH = DH + SIGMA   # x rows per half (67)

_NC_CACHE = {}


def _blur_matrix(g: np.ndarray) -> np.ndarray:
    # Dense 128x128 operator for a clamped (edge-padded) 1D blur along a
    # length-128 axis: A[i, j] = sum of g[k] over taps where clamp(i+k-3)==j.
    A = np.zeros((D, D), dtype=np.float64)
    for i in range(D):
        for k in range(K):
            j = min(max(i + k - SIGMA, 0), D - 1)
            A[i, j] += float(g[k])
    return A.astype(np.float32)


# in-chunk widths (cols) per half; each chunk feeds exactly one R1 block
CHUNKS = [2048] * 8
# R1 per-half evac blocks in 64-col tiles (16 tiles = 1024 f32 = 1 block)
SIZES1 = [16] * 8
# R2 per-half evac blocks in 128-col tiles; the B half tail-splits so the
# barrier into R3 drains on both engines in parallel
SIZES2 = [8] * 8
SIZES2B = [8] * 8
# last round over the merged volume; small head blocks start the store
# stream a little earlier
SIZES3 = [4, 4] + [8] * 15
PS_BUFS = 4


def _build():
    f16 = mybir.dt.float16
    f32 = mybir.dt.float32
    nc = bacc.Bacc("TRN2", target_bir_lowering=False, debug=False)
    x = nc.dram_tensor("x", [D, HW], f16, kind="ExternalInput")
    at = nc.dram_tensor("at", [D, D], f16, kind="ExternalInput")
    out = nc.dram_tensor("out", [D, HW], f16, kind="ExternalOutput")

    assert sum(CHUNKS) == HW
    assert sum(SIZES1) == D and sum(SIZES2) == sum(SIZES2B) == DH
    assert sum(SIZES3) == D

    # evac cost model (ns) for greedy engine balancing
    DVE_NS = lambda n: n * 1.0417 + 125.0
    ACT_NS = lambda n: n * 0.8333 + 185.0
    busy = {"v": 0.0, "s": 0.0}

    with tile.TileContext(nc) as tc:
        with tc.tile_pool(name="vol", bufs=1) as vol, \
             tc.tile_pool(name="cst", bufs=1) as cst, \
             tc.tile_pool(name="ps8", bufs=PS_BUFS, space="PSUM") as ps8:
            atta = cst.tile([XH, DH], f16)
            attb = cst.tile([XH, DH], f16)
            attf = cst.tile([D, D], f16)
            # constants ride the gpsimd SWDGE queue: they reach the DMA
            # engines before the first x chunk without displacing it
            nc.gpsimd.dma_start(atta[:], at[0:XH, 0:DH])
            nc.gpsimd.dma_start(attb[:], at[D - XH:D, DH:D])
            nc.gpsimd.dma_start(attf[:], at[:])

            v1a = vol.tile([XH, HW], f16)
            v1b = vol.tile([XH, HW], f16)
            v2 = vol.tile([D, HW], f16)   # [w | (dp h)]
            v3 = vol.tile([D, HW], f16)   # [h | (wp dp)]
            v4 = vol.tile([D, HW], f16)   # [dp | (wp hp)]

            c0 = 0
            for cw in CHUNKS:
                nc.sync.dma_start(v1a[:, c0:c0 + cw], x[0:XH, c0:c0 + cw])
                c0 += cw
            c0 = 0
            for cw in CHUNKS:
                nc.sync.dma_start(v1b[:, c0:c0 + cw], x[D - XH:D, c0:c0 + cw])
                c0 += cw

            def evac(dslc, src_ap, n, force=None):
                use_v = (busy["v"] + DVE_NS(n) <= busy["s"] + ACT_NS(n)
                         if force is None else force == "v")
                if use_v:
                    busy["v"] += DVE_NS(n)
                    nc.vector.tensor_copy(dslc, src_ap)
                else:
                    busy["s"] += ACT_NS(n)
                    nc.scalar.copy(dslc, src_ap)

            v2r = v2[:].rearrange("p (a t) -> p t a", a=D)  # [p, h, dp]
            v3r = v3[:].rearrange("p (a t) -> p t a", a=D)  # [p, dp, wp]

            def r1_block(src, atts, dlo, t0, bb, force=None):
                # blur d (on partitions, XH rows) for h tiles [t0, t0+bb);
                # out tile per h: [w, DH] at dp offset dlo
                pt = ps8.tile([D, bb * DH], f32)
                for s in range(bb):
                    t = t0 + s
                    nc.tensor.matmul(pt[:, s * DH:(s + 1) * DH],
                                     src[:, t * D:(t + 1) * D],
                                     atts[:], start=True, stop=True)
                evac(v2r[:, t0:t0 + bb, dlo:dlo + DH],
                     pt[:].rearrange("p (t a) -> p t a", t=bb),
                     bb * DH, force)

            def r2_block(dlo, t0, bb, force=None):
                # blur w (on partitions) for dp tiles [dlo+t0, dlo+t0+bb)
                pt = ps8.tile([D, bb * D], f32)
                for s in range(bb):
                    dp = dlo + t0 + s
                    nc.tensor.matmul(pt[:, s * D:(s + 1) * D],
                                     v2[:, dp * D:(dp + 1) * D],
                                     attf[:], start=True, stop=True)
                evac(v3r[:, dlo + t0:dlo + t0 + bb, :],
                     pt[:].rearrange("p (t a) -> p t a", t=bb),
                     bb * D, force)

            def blocks(sizes):
                t0, out_ = 0, []
                for bb in sizes:
                    out_.append((t0, bb))
                    t0 += bb
                return out_

            B1, B2 = blocks(SIZES1), blocks(SIZES2)

            # half A: R1A fully (paced by A's input chunks)
            for t0, bb in B1:
                r1_block(v1a, atta, 0, t0, bb)
            # R1B (paced by B's input chunks) with priority; R2A blocks
            # (ready once R1A is evacuated) fill the remaining slots
            merged = ([("1b",) + b for b in B1[:3]]
                      + [x for p in zip(
                          [("1b",) + b for b in B1[3:]],
                          [("2a",) + b for b in B2[:5]])
                         for x in p]
                      + [("2a",) + b for b in B2[5:]])
            for kind, t0, bb in merged:
                if kind == "1b":
                    r1_block(v1b, attb, DH, t0, bb)
                else:
                    r2_block(0, t0, bb)
            # R2B after R1B is fully evacuated
            B2B = blocks(SIZES2B)
            for i, (t0, bb) in enumerate(B2B):
                f = ("s" if i == len(B2B) - 2 else
                     "v" if i == len(B2B) - 1 else None)
                r2_block(DH, t0, bb, f)

            # R3: blur h (on partitions), full width; store chunks stream
            # out as soon as their evacs land
            for t0, bb in blocks(SIZES3):
                pt = ps8.tile([D, bb * D], f32)
                for s in range(bb):
                    wp = t0 + s
                    nc.tensor.matmul(pt[:, s * D:(s + 1) * D],
                                     v3[:, wp * D:(wp + 1) * D],
                                     attf[:], start=True, stop=True)
                lo, hi = t0 * D, (t0 + bb) * D
                evac(v4[:, lo:hi], pt[:], bb * D)
                if hi % 1024 == 0:
                    nc.sync.dma_start(out[:, hi - 1024:hi],
                                      v4[:, hi - 1024:hi])
    nc.finalize()
    return nc


def kernel(x, g, sigma):
    x = np.asarray(x, dtype=np.float32)
    if "nc" not in _NC_CACHE:
        _NC_CACHE["nc"] = _build()
    nc = _NC_CACHE["nc"]
    AT = np.ascontiguousarray(_blur_matrix(np.asarray(g, np.float64)).T
                              ).astype(np.float16)
    slabs = x.reshape(N_CORES, D, HW).astype(np.float16)
    in_maps = [{"x": np.ascontiguousarray(slabs[i]), "at": AT}
               for i in range(N_CORES)]
    res = run_bass_kernel_spmd(nc, in_maps, core_ids=list(range(N_CORES)))
    outs = np.stack([res.results[i]["out"] for i in range(N_CORES)])
    # device emits [d, w, h]; swap the last two axes back to [d, h, w]
    outs = outs.reshape(N_CORES, D, W, H).astype(np.float32)
    return np.ascontiguousarray(outs.transpose(0, 1, 3, 2)
                                ).reshape(2, 4, D, H, W)
